# revision 1
# baseline (speedup 1.0000x reference)
"""Multi-head self-attention (no mask) on 8 TRN2 NeuronCores.

Sharding: tensor-parallel over heads (2 heads/core) for QKV + attention,
then an AllToAll re-shards to row-parallel for the output projection.

Per-core dataflow:
  A) qT,kT = W{q,k}_c @ x.T (transposed layout), v = x @ Wv_c.T (natural).
     Inputs cast to bf16 on DVE; matmuls in bf16, f32 PSUM accum.
  B) per (head, batch, q-chunk): scoresT = k q^T, expT = exp(scale*scoresT)
     [no max-subtraction: scores are O(5)], attnT_unnorm = v^T expT via PE,
     softmax denominators via a DVE pairwise-add tree over expT k-tiles
     + one ones^T matmul, attnT = attnT_unnorm * broadcast(1/sums).
     Chunks are software-pipelined one deep (scores/exp of chunk i+1 issue
     ahead of PV/normalize of chunk i) so ACT never starves the PE.
     One AllToAll per head; payloads in bf16.
  C) out_rows = attn_rows @ Wo.T + bo (bf16 matmuls, f32 accum + bias).
     The first few (n,m) tiles pre-run their even k-tiles (head-0 features,
     available after the first AllToAll) to overlap the second AllToAll.

A single tagged PSUM pool (8 banks shared across phases) avoids
phase-boundary serialization on PSUM.
"""

import numpy as np

import concourse.bass as bass
import concourse.tile as tile
from concourse import bacc, mybir
from concourse.bass_utils import run_bass_kernel_spmd

F32 = mybir.dt.float32
F32R = mybir.dt.float32r
BF16 = mybir.dt.bfloat16

B, S, H = 2, 2048, 2048
NH, HD = 16, 128
NC = 8
BS = B * S          # 4096 rows total
FL = H // NC        # 256 features per core (2 heads)
HL = NH // NC       # 2 heads per core
RPC = BS // NC      # 512 output rows per core
K16 = H // 128      # 16 contraction tiles
CW = 512            # phase-A row-chunk width
NCHUNK = BS // CW   # 8
QC = 512            # attention q-chunk width
SCALE = 1.0 / float(np.sqrt(HD))

_CACHED = None


def _build():
    nc = bacc.Bacc("TRN2", target_bir_lowering=False, debug=False, num_devices=NC)

    xT_d = nc.dram_tensor("xT", [H, BS], F32, kind="ExternalInput")
    wqT_d = nc.dram_tensor("wqT", [H, FL], F32, kind="ExternalInput")
    wkT_d = nc.dram_tensor("wkT", [H, FL], F32, kind="ExternalInput")
    wvT_d = nc.dram_tensor("wvT", [H, FL], F32, kind="ExternalInput")
    bq_d = nc.dram_tensor("bq", [128, HL], F32, kind="ExternalInput")
    bk_d = nc.dram_tensor("bk", [128, HL], F32, kind="ExternalInput")
    bv_d = nc.dram_tensor("bv_bc", [128, FL], F32, kind="ExternalInput")
    woT_d = nc.dram_tensor("woT", [H, H], F32, kind="ExternalInput")
    bo_d = nc.dram_tensor("bo_bc", [128, H], F32, kind="ExternalInput")
    onesb_d = nc.dram_tensor("ones_bf", [128, 128], BF16, kind="ExternalInput")
    out_d = nc.dram_tensor("out", [RPC, H], F32, kind="ExternalOutput")

    with tile.TileContext(nc) as tc:
        with (
            tc.tile_pool(name="consts", bufs=1) as cstp,
            tc.tile_pool(name="dram", bufs=1, space="DRAM") as dp,
            tc.tile_pool(name="stg", bufs=1) as stg,
            tc.tile_pool(name="woPre", bufs=1) as wcp,
            tc.tile_pool(name="aTp", bufs=1) as atp,
            tc.tile_pool(name="psum", bufs=1, space="PSUM") as pp,
        ):
            ones_bf = cstp.tile([128, 128], BF16)
            nc.sync.dma_start(ones_bf[:], onesb_d.ap()[:])

            a2a_in = [dp.tile([NC, 128, RPC], BF16, name=f"a2a_in{h}") for h in range(HL)]
            a2a_out = [dp.tile([NC, 128, RPC], BF16, name=f"a2a_out{h}") for h in range(HL)]
            aT = atp.tile([128, K16 * RPC], BF16)

            def cast_load(dst_slice, src_ap, width):
                """DMA f32 -> small staging, DVE-cast into bf16 dst slice."""
                src = stg.tile([128, CW], F32, tag="stg", bufs=6)
                nc.sync.dma_start(src[:, :width], src_ap)
                nc.vector.tensor_copy(dst_slice, src[:, :width])

            with tc.tile_pool(name="qkv", bufs=1) as qkvp:
                qT_sb = qkvp.tile([128, HL * BS], BF16)
                kT_sb = qkvp.tile([128, HL * BS], BF16)
                v_sb = qkvp.tile([128, (BS // 128) * FL], BF16)

                # ---------------- Phase A: QKV projections ----------------
                with (
                    tc.tile_pool(name="wgt", bufs=1) as wp,
                    tc.tile_pool(name="xbf", bufs=2) as xbp,
                ):
                    def load_w(dram):
                        dst = wp.tile([128, K16 * FL], BF16, tag=f"w_{dram.name}")
                        for k in range(K16):
                            cast_load(
                                dst[:, k * FL:(k + 1) * FL],
                                dram.ap()[k * 128:(k + 1) * 128, :],
                                FL,
                            )
                        return dst

                    def load_x(c):
                        dst = xbp.tile([128, K16 * CW], BF16, tag="xbf")
                        for k in range(K16):
                            cast_load(
                                dst[:, k * CW:(k + 1) * CW],
                                xT_d.ap()[k * 128:(k + 1) * 128,
                                          c * CW:(c + 1) * CW],
                                CW,
                            )
                        return dst

                    wq_sb = wp.tile([128, K16 * FL], BF16, tag="w_wqT")
                    xc0 = xbp.tile([128, K16 * CW], BF16, tag="xbf")
                    for k in range(K16):
                        cast_load(
                            wq_sb[:, k * FL:(k + 1) * FL],
                            wqT_d.ap()[k * 128:(k + 1) * 128, :],
                            FL,
                        )
                        cast_load(
                            xc0[:, k * CW:(k + 1) * CW],
                            xT_d.ap()[k * 128:(k + 1) * 128, :CW],
                            CW,
                        )
                    wk_sb = load_w(wkT_d)
                    wv_sb = load_w(wvT_d)

                    bq_sb = wp.tile([128, HL], F32)
                    nc.sync.dma_start(bq_sb[:], bq_d.ap()[:])
                    bk_sb = wp.tile([128, HL], F32)
                    nc.sync.dma_start(bk_sb[:], bk_d.ap()[:])
                    bv_sb = wp.tile([128, FL], F32)
                    nc.sync.dma_start(bv_sb[:], bv_d.ap()[:])

                    for c in range(NCHUNK):
                        xc = xc0 if c == 0 else load_x(c)
                        for w_sb, b_sb, dst in (
                            (wq_sb, bq_sb, qT_sb),
                            (wk_sb, bk_sb, kT_sb),
                        ):
                            for m in range(HL):
                                ps = pp.tile([128, CW], F32, tag="pss", bufs=4)
                                for k in range(K16):
                                    nc.tensor.matmul(
                                        ps[:],
                                        w_sb[:, k * FL + m * 128:
                                             k * FL + (m + 1) * 128],
                                        xc[:, k * CW:(k + 1) * CW],
                                        start=(k == 0),
                                        stop=(k == K16 - 1),
                                    )
                                nc.vector.tensor_scalar_add(
                                    dst[:, m * BS + c * CW: m * BS + (c + 1) * CW],
                                    ps[:],
                                    b_sb[:, m:m + 1],
                                )
                        for m2 in range(CW // 128):
                            ps = pp.tile([128, FL], F32, tag="psa", bufs=2)
                            for k in range(K16):
                                nc.tensor.matmul(
                                    ps[:],
                                    xc[:, k * CW + m2 * 128: k * CW + (m2 + 1) * 128],
                                    wv_sb[:, k * FL:(k + 1) * FL],
                                    start=(k == 0),
                                    stop=(k == K16 - 1),
                                )
                            i = c * (CW // 128) + m2
                            nc.vector.tensor_add(
                                v_sb[:, i * FL:(i + 1) * FL], ps[:], bv_sb[:]
                            )

                # prefetch Wo (cast to bf16) during attention
                won_tiles = {}
                for n in range(2):
                    won = wcp.tile([128, K16 * 512], BF16, tag="won", bufs=2)
                    for k in range(K16):
                        cast_load(
                            won[:, k * 512:(k + 1) * 512],
                            woT_d.ap()[k * 128:(k + 1) * 128,
                                       n * 512:(n + 1) * 512],
                            512,
                        )
                    won_tiles[n] = won

                # ---------------- Phase B: attention (1-deep pipelined) ----
                with (
                    tc.tile_pool(name="expp", bufs=3) as ep,
                    tc.tile_pool(name="tree", bufs=2) as trp,
                    tc.tile_pool(name="attp", bufs=2) as ap_,
                    tc.tile_pool(name="recp", bufs=2) as rp,
                ):
                    chunks = [
                        (h, b, qc)
                        for h in range(HL)
                        for b in range(B)
                        for qc in range(S // QC)
                    ]
                    pend = {}

                    def emit_scores(i):
                        h, b, qc = chunks[i]
                        base = h * BS + b * S
                        expT = ep.tile([128, K16 * QC], BF16, tag="expT")
                        for km in range(K16):
                            pss = pp.tile([128, QC], F32, tag="pss", bufs=4)
                            nc.tensor.matmul(
                                pss[:],
                                kT_sb[:, base + km * 128: base + (km + 1) * 128],
                                qT_sb[:, base + qc * QC: base + (qc + 1) * QC],
                                start=True,
                                stop=True,
                            )
                            nc.scalar.activation(
                                expT[:, km * QC:(km + 1) * QC],
                                pss[:],
                                mybir.ActivationFunctionType.Exp,
                                scale=SCALE,
                            )
                        pend[i] = expT

                    def emit_tree(i):
                        expT = pend[i]
                        s1 = trp.tile([128, 8 * QC], BF16, tag="s1")
                        nc.vector.tensor_add(s1[:], expT[:, :8 * QC], expT[:, 8 * QC:])
                        s2 = trp.tile([128, 4 * QC], BF16, tag="s2")
                        nc.vector.tensor_add(s2[:], s1[:, :4 * QC], s1[:, 4 * QC:])
                        s3 = trp.tile([128, 2 * QC], BF16, tag="s3")
                        nc.vector.tensor_add(s3[:], s2[:, :2 * QC], s2[:, 2 * QC:])
                        s4 = trp.tile([128, QC], BF16, tag="s4")
                        nc.vector.tensor_add(s4[:], s3[:, :QC], s3[:, QC:])
                        pend[i] = (pend[i], s4)

                    def emit_pv(i):
                        h, b, qc = chunks[i]
                        dest = b * (S // QC) + qc
                        expT, s4 = pend.pop(i)
                        psa = pp.tile([128, QC], F32, tag="psa", bufs=2)
                        for km in range(K16):
                            nc.tensor.matmul(
                                psa[:],
                                v_sb[:, (16 * b + km) * FL + h * 128:
                                     (16 * b + km) * FL + (h + 1) * 128],
                                expT[:, km * QC:(km + 1) * QC],
                                start=(km == 0),
                                stop=(km == K16 - 1),
                            )
                        pssum = pp.tile([1, QC], F32, tag="pssum", bufs=1)
                        nc.tensor.matmul(
                            pssum[:1, :], ones_bf[:, :1], s4[:],
                            start=True, stop=True,
                        )
                        recip = rp.tile([1, QC], BF16, tag="recip")
                        with nc.allow_low_precision(
                            reason="bf16 recip; rb rounds to bf16 anyway"
                        ):
                            nc.vector.reciprocal(recip[:1, :], pssum[:1, :])
                        psb = pp.tile([128, QC], F32, tag="psb", bufs=1)
                        nc.tensor.matmul(
                            psb[:], ones_bf[:1, :], recip[:1, :],
                            start=True, stop=True,
                        )
                        rb = rp.tile([128, QC], BF16, tag="rb")
                        nc.vector.tensor_copy(rb[:], psb[:])
                        att = ap_.tile([128, QC], BF16, tag="att")
                        nc.vector.tensor_mul(att[:], psa[:], rb[:])
                        nc.gpsimd.dma_start(a2a_in[h][dest, :, :], att[:])

                    n_chunks = len(chunks)
                    for i in range(n_chunks):
                        emit_scores(i)
                        emit_tree(i)
                        emit_pv(i)
                        h_done, b_done, qc_done = chunks[i]
                        if (b_done, qc_done) == (B - 1, S // QC - 1):
                            nc.gpsimd.collective_compute(
                                "AllToAll",
                                mybir.AluOpType.bypass,
                                ins=[a2a_in[h_done].opt()],
                                outs=[a2a_out[h_done].opt()],
                                replica_groups=[list(range(NC))],
                            )

            # ---------------- Phase C: output projection ----------------
            with (
                tc.tile_pool(name="boC", bufs=1) as bcp,
                tc.tile_pool(name="outC", bufs=3) as ocp,
            ):
                for g in range(K16):
                    nc.sync.dma_start(
                        aT[:, g * RPC:(g + 1) * RPC],
                        a2a_out[g % 2][g // 2, :, :],
                    )
                bo_sb = bcp.tile([128, H], F32)
                nc.sync.dma_start(bo_sb[:], bo_d.ap()[:])
                PSO_TAGS = [("pss", 4), ("psa", 2), ("psb", 1)]
                for idx, (n, m) in enumerate(
                    (n, m) for n in range(H // 512) for m in range(RPC // 128)
                ):
                    if n in won_tiles:
                        won = won_tiles[n]
                    else:
                        won = wcp.tile([128, K16 * 512], BF16, tag="won", bufs=2)
                        for k in range(K16):
                            cast_load(
                                won[:, k * 512:(k + 1) * 512],
                                woT_d.ap()[k * 128:(k + 1) * 128,
                                           n * 512:(n + 1) * 512],
                                512,
                            )
                        won_tiles[n] = won
                    tag, bufs = PSO_TAGS[idx % 3]
                    pso = pp.tile([128, 512], F32, tag=tag, bufs=bufs)
                    for k in range(K16):
                        nc.tensor.matmul(
                            pso[:],
                            aT[:, k * RPC + m * 128: k * RPC + (m + 1) * 128],
                            won[:, k * 512:(k + 1) * 512],
                            start=(k == 0),
                            stop=(k == K16 - 1),
                        )
                    ot = ocp.tile([128, 512], F32, tag="ot")
                    nc.vector.tensor_add(
                        ot[:], pso[:], bo_sb[:, n * 512:(n + 1) * 512]
                    )
                    nc.sync.dma_start(
                        out_d.ap()[m * 128:(m + 1) * 128, n * 512:(n + 1) * 512],
                        ot[:],
                    )

    nc.compile()
    return nc


def _get_nc():
    global _CACHED
    if _CACHED is None:
        _CACHED = _build()
    return _CACHED


def _prep_in_maps(x, Wq, bq, Wk, bk, Wv, bv, Wo, bo):
    import ml_dtypes

    xT = np.ascontiguousarray(x.reshape(BS, H).T)
    woT = np.ascontiguousarray(Wo.T)
    bo_bc = np.ascontiguousarray(np.broadcast_to(bo, (128, H)))
    ones_bf = np.ones((128, 128), ml_dtypes.bfloat16)
    in_maps = []
    for c in range(NC):
        sl = slice(FL * c, FL * (c + 1))
        in_maps.append(
            {
                "xT": xT,
                "wqT": np.ascontiguousarray(Wq[sl, :].T),
                "wkT": np.ascontiguousarray(Wk[sl, :].T),
                "wvT": np.ascontiguousarray(Wv[sl, :].T),
                "bq": np.ascontiguousarray(bq[sl].reshape(HL, 128).T),
                "bk": np.ascontiguousarray(bk[sl].reshape(HL, 128).T),
                "bv_bc": np.ascontiguousarray(np.broadcast_to(bv[sl], (128, FL))),
                "woT": woT,
                "bo_bc": bo_bc,
                "ones_bf": ones_bf,
            }
        )
    return in_maps


def run(in_maps, trace=False):
    nc = _get_nc()
    return run_bass_kernel_spmd(nc, in_maps, core_ids=list(range(NC)), trace=trace)


def kernel(x, Wq, bq, Wk, bk, Wv, bv, Wo, bo):
    args = [np.asarray(a, dtype=np.float32) for a in (x, Wq, bq, Wk, bk, Wv, bv, Wo, bo)]
    in_maps = _prep_in_maps(*args)
    res = run(in_maps)
    out = np.concatenate([res.results[c]["out"] for c in range(NC)], axis=0)
    return out.reshape(B, S, H)



# revision 7
# speedup vs baseline: 1.0947x; 1.0947x over previous
"""Multi-head self-attention (no mask) on 8 TRN2 NeuronCores.

Sharding: tensor-parallel over heads (2 heads/core) for QKV + attention,
then an AllToAll re-shards to row-parallel for the output projection.

v2 structure (fused schedule, all inputs pre-cast to bf16 on host):
  Stage 1: QKV projections for row-chunks 0-3 (batch 0).
  Stage 2: projections for chunks 4-7 (batch 1) interleaved with
     attention chunks (h0, b0, *) so the PE covers ACT's exp latency.
  Stage 3: remaining 12 attention chunks; AllToAll(h0) fires 1/3 in;
     out-projection even-k-slab pre-runs (stashed to SBUF bf16 partials)
     fill the AllToAll(h1) window.
  Stage 4: odd-k-slab accumulation + partial add + bias, stream out.

Attention chunk: scores into a [128,2048] 4-bank PSUM tile (4 MMs), one
wide exp ACTIVATE per quad; PV accumulates v^T expT; softmax denominators
via DVE pairwise tree + a ones[128x128] matmul that broadcasts the sums
to all partitions in one shot; reciprocal_approx_fast + one DVE mul
normalizes. No max-subtraction (scores are O(5)).
"""

import numpy as np

import concourse.bass as bass
import concourse.tile as tile
from concourse import bacc, mybir
from concourse.bass_utils import run_bass_kernel_spmd

F32 = mybir.dt.float32
BF16 = mybir.dt.bfloat16

B, S, H = 2, 2048, 2048
NH, HD = 16, 128
NC = 8
BS = B * S          # 4096 rows total
FL = H // NC        # 256 features per core (2 heads)
HL = NH // NC       # 2 heads per core
RPC = BS // NC      # 512 output rows per core
K16 = H // 128      # 16 contraction tiles
CW = 512            # row-chunk width
QC = 512            # attention q-chunk width
SCALE = 1.0 / float(np.sqrt(HD))

_CACHED = None


def _build():
    nc = bacc.Bacc("TRN2", target_bir_lowering=False, debug=False, num_devices=NC)

    xT_d = nc.dram_tensor("xT_t", [128, K16, BS], BF16, kind="ExternalInput")
    wqT_d = nc.dram_tensor("wqT_t", [128, K16, FL], BF16, kind="ExternalInput")
    wkT_d = nc.dram_tensor("wkT_t", [128, K16, FL], BF16, kind="ExternalInput")
    wvT_d = nc.dram_tensor("wvT_t", [128, K16, FL], BF16, kind="ExternalInput")
    bq_d = nc.dram_tensor("bq", [128, HL], F32, kind="ExternalInput")
    bk_d = nc.dram_tensor("bk", [128, HL], F32, kind="ExternalInput")
    bv_d = nc.dram_tensor("bv_bc", [128, FL], F32, kind="ExternalInput")
    woT_d = nc.dram_tensor("woT_t", [128, K16, H], BF16, kind="ExternalInput")
    bo_d = nc.dram_tensor("bo_bc", [128, H], BF16, kind="ExternalInput")
    onesb_d = nc.dram_tensor("ones_bf", [128, 128], BF16, kind="ExternalInput")
    out_d = nc.dram_tensor("out", [RPC, H], F32, kind="ExternalOutput")

    with tile.TileContext(nc) as tc:
        with (
            tc.tile_pool(name="consts", bufs=1) as cstp,
            tc.tile_pool(name="dram", bufs=1, space="DRAM") as dp,
            tc.tile_pool(name="qkv", bufs=1) as qkvp,
            tc.tile_pool(name="wo01", bufs=1) as wop,
            tc.tile_pool(name="attn", bufs=1) as ap_,
            tc.tile_pool(name="psum", bufs=1, space="PSUM") as pp,
        ):
            ones_bf = cstp.tile([128, 128], BF16)
            nc.sync.dma_start(ones_bf[:], onesb_d.ap()[:])
            bq_sb = cstp.tile([128, HL], F32)
            nc.sync.dma_start(bq_sb[:], bq_d.ap()[:])
            bk_sb = cstp.tile([128, HL], F32)
            nc.sync.dma_start(bk_sb[:], bk_d.ap()[:])
            bv_sb = cstp.tile([128, FL], F32)
            nc.sync.dma_start(bv_sb[:], bv_d.ap()[:])
            bo_sb = cstp.tile([128, H], BF16)
            nc.sync.dma_start(bo_sb[:], bo_d.ap()[:])

            a2a_in = [dp.tile([NC, 128, RPC], BF16, name=f"a2a_in{h}") for h in range(HL)]
            a2a_out = [dp.tile([NC, 128, RPC], BF16, name=f"a2a_out{h}") for h in range(HL)]

            qT_sb = qkvp.tile([128, HL * BS], BF16)
            kT_sb = qkvp.tile([128, HL * BS], BF16)
            v_sb = qkvp.tile([128, (BS // 128) * FL], BF16)

            won_tiles = {}

            def load_wo(pool, n):
                won = pool.tile([128, K16 * 512], BF16, tag="won", bufs=2)
                nc.sync.dma_start(won[:], woT_d.ap()[:, :, n * 512:(n + 1) * 512])
                won_tiles[n] = won

            # ---------------- projection building blocks ----------------
            def emit_proj_qk(w_sb, b_sb, dst, c, m):
                """One [128 feats x 512 rows] output block of q/k for chunk c."""
                xc = x_tiles[c]
                ps = pp.tile([128, CW], F32, tag="pa", bufs=3)
                for k in range(K16):
                    nc.tensor.matmul(
                        ps[:],
                        w_sb[:, k * FL + m * 128: k * FL + (m + 1) * 128],
                        xc[:, k * CW:(k + 1) * CW],
                        start=(k == 0),
                        stop=(k == K16 - 1),
                    )
                nc.vector.tensor_scalar_add(
                    dst[:, m * BS + c * CW: m * BS + (c + 1) * CW],
                    ps[:],
                    b_sb[:, m:m + 1],
                )

            def emit_proj_v(c, m2):
                """One [128 rows x 256 feats] block of v for chunk c."""
                xc = x_tiles[c]
                ps = pp.tile([128, CW], F32, tag="pa", bufs=3)
                for k in range(K16):
                    nc.tensor.matmul(
                        ps[:, :FL],
                        xc[:, k * CW + m2 * 128: k * CW + (m2 + 1) * 128],
                        wv_sb[:, k * FL:(k + 1) * FL],
                        start=(k == 0),
                        stop=(k == K16 - 1),
                    )
                i = c * (CW // 128) + m2
                nc.vector.tensor_add(
                    v_sb[:, i * FL:(i + 1) * FL], ps[:, :FL], bv_sb[:]
                )

            # ---------------- attention building blocks ----------------
            pend = {}

            def emit_scores_quad(key, h, b, qc, quad):
                """4 score MMs into a 4-bank PSUM tile + one wide exp."""
                base = h * BS + b * S
                if quad == 0:
                    pend[key] = ap_.tile(
                        [128, K16 * QC], BF16, tag="expT", bufs=2, name="expT"
                    )
                expT = pend[key]
                pss = pp.tile([128, 2048], F32, tag="pss", bufs=1)
                for j in range(4):
                    km = quad * 4 + j
                    nc.tensor.matmul(
                        pss[:, j * QC:(j + 1) * QC],
                        kT_sb[:, base + km * 128: base + (km + 1) * 128],
                        qT_sb[:, base + qc * QC: base + (qc + 1) * QC],
                        start=True,
                        stop=True,
                    )
                nc.scalar.activation(
                    expT[:, quad * 4 * QC:(quad + 1) * 4 * QC],
                    pss[:],
                    mybir.ActivationFunctionType.Exp,
                    scale=SCALE,
                )

            def emit_pv_norm(key, h, b, qc):
                """PV accumulation, denominator tree, normalize, ship."""
                dest = b * (S // QC) + qc
                expT = pend.pop(key)
                psa = pp.tile([128, QC], F32, tag="pa", bufs=3)
                for km in range(K16):
                    nc.tensor.matmul(
                        psa[:],
                        v_sb[:, (16 * b + km) * FL + h * 128:
                             (16 * b + km) * FL + (h + 1) * 128],
                        expT[:, km * QC:(km + 1) * QC],
                        start=(km == 0),
                        stop=(km == K16 - 1),
                    )
                s2 = ap_.tile([128, 4 * QC], BF16, tag="s2", bufs=1)
                nc.vector.tensor_add(s2[:], expT[:, :4 * QC], expT[:, 4 * QC:8 * QC])
                nc.vector.tensor_add(s2[:], s2[:], expT[:, 8 * QC:12 * QC])
                nc.vector.tensor_add(s2[:], s2[:], expT[:, 12 * QC:])
                s3 = ap_.tile([128, 2 * QC], BF16, tag="s3", bufs=1)
                nc.vector.tensor_add(s3[:], s2[:, :2 * QC], s2[:, 2 * QC:])
                s4 = ap_.tile([128, QC], BF16, tag="s4", bufs=2)
                nc.vector.tensor_add(s4[:], s3[:, :QC], s3[:, QC:])
                # broadcast column sums to all 128 partitions in one MM
                psum_bc = pp.tile([128, QC], F32, tag="pbc", bufs=1)
                nc.tensor.matmul(psum_bc[:], ones_bf[:], s4[:], start=True, stop=True)
                rb = ap_.tile([128, QC], F32, tag="rb", bufs=2)
                nc.vector.reciprocal_approx_fast(rb[:], psum_bc[:])
                att = ap_.tile([128, QC], BF16, tag="att", bufs=2)
                nc.vector.tensor_mul(att[:], psa[:], rb[:])
                nc.gpsimd.dma_start(a2a_in[h][dest, :, :], att[:])

            def emit_att_chunk(h, b, qc):
                key = (h, b, qc)
                for quad in range(4):
                    emit_scores_quad(key, h, b, qc, quad)
                emit_pv_norm(key, h, b, qc)

            # ---------------- stages 1+2 (x/w pools open) ----------------
            with (
                tc.tile_pool(name="wgt", bufs=1) as wp,
                tc.tile_pool(name="xbf", bufs=1) as xbp,
            ):
                wq_sb = wp.tile([128, K16 * FL], BF16, tag="wq")
                nc.sync.dma_start(wq_sb[:], wqT_d.ap()[:])
                wk_sb = wp.tile([128, K16 * FL], BF16, tag="wk")
                nc.sync.dma_start(wk_sb[:], wkT_d.ap()[:])
                wv_sb = wp.tile([128, K16 * FL], BF16, tag="wv")
                nc.sync.dma_start(wv_sb[:], wvT_d.ap()[:])

                x_tiles = {}

                def load_x(c):
                    xc = xbp.tile([128, K16 * CW], BF16, tag="x", bufs=2)
                    nc.sync.dma_start(xc[:], xT_d.ap()[:, :, c * CW:(c + 1) * CW])
                    x_tiles[c] = xc

                # Stage 1: chunks 0-3 (batch 0)
                load_x(0)
                for c in range(4):
                    if c + 1 < 4:
                        load_x(c + 1)
                    for m in range(HL):
                        emit_proj_qk(wq_sb, bq_sb, qT_sb, c, m)
                        emit_proj_qk(wk_sb, bk_sb, kT_sb, c, m)
                    for m2 in range(CW // 128):
                        emit_proj_v(c, m2)

                # Stage 2: chunks 4-7 interleaved with attention (h0, b0, *)
                load_x(4)
                for i in range(4):
                    c = 4 + i
                    if c + 1 < 8:
                        load_x(c + 1)
                    if i < 2:
                        load_wo(wop, i)
                    key = (0, 0, i)
                    emit_scores_quad(key, 0, 0, i, 0)
                    emit_proj_qk(wq_sb, bq_sb, qT_sb, c, 0)
                    emit_scores_quad(key, 0, 0, i, 1)
                    emit_proj_qk(wq_sb, bq_sb, qT_sb, c, 1)
                    emit_scores_quad(key, 0, 0, i, 2)
                    emit_proj_qk(wk_sb, bk_sb, kT_sb, c, 0)
                    emit_scores_quad(key, 0, 0, i, 3)
                    emit_proj_qk(wk_sb, bk_sb, kT_sb, c, 1)
                    emit_pv_norm(key, 0, 0, i)
                    for m2 in range(CW // 128):
                        emit_proj_v(c, m2)

            # ---------------- stages 3+4 ----------------
            with (
                tc.tile_pool(name="wo23", bufs=1) as wop2,
                tc.tile_pool(name="aTp", bufs=1) as atp,
                tc.tile_pool(name="cpart", bufs=1) as cpp,
                tc.tile_pool(name="outC", bufs=1) as ocp,
            ):
                aT = atp.tile([128, K16 * RPC], BF16)
                partials = cpp.tile([128, 16 * 512], BF16)
                ctiles = [(n, m) for n in range(4) for m in range(4)]

                def emit_c_even(t):
                    n, m = ctiles[t]
                    won = won_tiles[n]
                    pso = pp.tile([128, 512], F32, tag="pa", bufs=3)
                    for j in range(8):
                        k = 2 * j
                        nc.tensor.matmul(
                            pso[:],
                            aT[:, k * RPC + m * 128: k * RPC + (m + 1) * 128],
                            won[:, k * 512:(k + 1) * 512],
                            start=(j == 0),
                            stop=(j == 7),
                        )
                    # stash evens + bias as a bf16 partial
                    nc.vector.tensor_add(
                        partials[:, t * 512:(t + 1) * 512],
                        pso[:],
                        bo_sb[:, n * 512:(n + 1) * 512],
                    )

                def emit_c_odd(t):
                    n, m = ctiles[t]
                    won = won_tiles[n]
                    pso = pp.tile([128, 512], F32, tag="pa", bufs=3)
                    for j in range(8):
                        k = 2 * j + 1
                        nc.tensor.matmul(
                            pso[:],
                            aT[:, k * RPC + m * 128: k * RPC + (m + 1) * 128],
                            won[:, k * 512:(k + 1) * 512],
                            start=(j == 0),
                            stop=(j == 7),
                        )
                    ot = ocp.tile([128, 512], F32, tag="ot", bufs=3)
                    nc.vector.tensor_add(
                        ot[:], pso[:], partials[:, t * 512:(t + 1) * 512]
                    )
                    nc.sync.dma_start(
                        out_d.ap()[m * 128:(m + 1) * 128, n * 512:(n + 1) * 512],
                        ot[:],
                    )

                # Stage 3: att (h0,b1,*) -> A2A(h0); (h1,b0,*), (h1,b1,*)
                # -> A2A(h1); C-even pre-runs interleave once aT evens land.
                for qc in range(4):
                    if qc < 2:
                        load_wo(wop2, 2 + qc)
                    emit_att_chunk(0, 1, qc)
                nc.gpsimd.collective_compute(
                    "AllToAll",
                    mybir.AluOpType.bypass,
                    ins=[a2a_in[0].opt()],
                    outs=[a2a_out[0].opt()],
                    replica_groups=[list(range(NC))],
                )
                for g in range(0, K16, 2):
                    nc.sync.dma_start(
                        aT[:, g * RPC:(g + 1) * RPC], a2a_out[0][g // 2, :, :]
                    )
                for j, (b, qc) in enumerate(
                    [(b, qc) for b in range(B) for qc in range(4)]
                ):
                    emit_att_chunk(1, b, qc)
                    if j >= 4:
                        emit_c_even(2 * (j - 4))
                        emit_c_even(2 * (j - 4) + 1)
                nc.gpsimd.collective_compute(
                    "AllToAll",
                    mybir.AluOpType.bypass,
                    ins=[a2a_in[1].opt()],
                    outs=[a2a_out[1].opt()],
                    replica_groups=[list(range(NC))],
                )
                for t in range(8, 16):
                    emit_c_even(t)
                for g in range(1, K16, 2):
                    nc.sync.dma_start(
                        aT[:, g * RPC:(g + 1) * RPC], a2a_out[1][g // 2, :, :]
                    )

                # Stage 4: odd halves + combine
                for t in range(16):
                    emit_c_odd(t)

    nc.compile()
    return nc


def _get_nc():
    global _CACHED
    if _CACHED is None:
        _CACHED = _build()
    return _CACHED


def _prep_in_maps(x, Wq, bq, Wk, bk, Wv, bv, Wo, bo):
    import ml_dtypes

    bf = ml_dtypes.bfloat16

    def tile_kmaj(a2d):
        # [H, N] -> [128, K16, N] with row r = k*128 + p
        h, n = a2d.shape
        return np.ascontiguousarray(
            a2d.reshape(K16, 128, n).transpose(1, 0, 2).astype(bf)
        )

    xT_t = tile_kmaj(x.reshape(BS, H).T)
    woT_t = tile_kmaj(Wo.T)
    bo_bc = np.ascontiguousarray(np.broadcast_to(bo, (128, H)).astype(bf))
    ones_bf = np.ones((128, 128), bf)
    in_maps = []
    for c in range(NC):
        sl = slice(FL * c, FL * (c + 1))
        in_maps.append(
            {
                "xT_t": xT_t,
                "wqT_t": tile_kmaj(np.ascontiguousarray(Wq[sl, :].T)),
                "wkT_t": tile_kmaj(np.ascontiguousarray(Wk[sl, :].T)),
                "wvT_t": tile_kmaj(np.ascontiguousarray(Wv[sl, :].T)),
                "bq": np.ascontiguousarray(bq[sl].reshape(HL, 128).T),
                "bk": np.ascontiguousarray(bk[sl].reshape(HL, 128).T),
                "bv_bc": np.ascontiguousarray(np.broadcast_to(bv[sl], (128, FL))),
                "woT_t": woT_t,
                "bo_bc": bo_bc,
                "ones_bf": ones_bf,
            }
        )
    return in_maps


def run(in_maps, trace=False):
    nc = _get_nc()
    return run_bass_kernel_spmd(nc, in_maps, core_ids=list(range(NC)), trace=trace)


def kernel(x, Wq, bq, Wk, bk, Wv, bv, Wo, bo):
    args = [np.asarray(a, dtype=np.float32) for a in (x, Wq, bq, Wk, bk, Wv, bv, Wo, bo)]
    in_maps = _prep_in_maps(*args)
    res = run(in_maps)
    out = np.concatenate([res.results[c]["out"] for c in range(NC)], axis=0)
    return out.reshape(B, S, H)


# revision 12
# speedup vs baseline: 1.1187x; 1.0219x over previous
"""Multi-head self-attention (no mask) on 8 TRN2 NeuronCores.

Sharding: tensor-parallel over heads (2 heads/core) for QKV + attention,
then an AllToAll re-shards to row-parallel for the output projection.

v2 structure (fused schedule, all inputs pre-cast to bf16 on host):
  Stage 1: QKV projections for row-chunks 0-3 (batch 0).
  Stage 2: projections for chunks 4-7 (batch 1) interleaved with
     attention chunks (h0, b0, *) so the PE covers ACT's exp latency.
  Stage 3: remaining 12 attention chunks; AllToAll(h0) fires 1/3 in;
     out-projection even-k-slab pre-runs (stashed to SBUF bf16 partials)
     fill the AllToAll(h1) window.
  Stage 4: odd-k-slab accumulation + partial add + bias, stream out.

Attention chunk: scores into a [128,2048] 4-bank PSUM tile (4 MMs), one
wide exp ACTIVATE per quad; PV accumulates v^T expT; softmax denominators
via DVE pairwise tree + a ones[128x128] matmul that broadcasts the sums
to all partitions in one shot; reciprocal_approx_fast + one DVE mul
normalizes. No max-subtraction (scores are O(5)).
"""

import numpy as np

import concourse.bass as bass
import concourse.tile as tile
from concourse import bacc, mybir
from concourse.bass_utils import run_bass_kernel_spmd

F32 = mybir.dt.float32
BF16 = mybir.dt.bfloat16

B, S, H = 2, 2048, 2048
NH, HD = 16, 128
NC = 8
BS = B * S          # 4096 rows total
FL = H // NC        # 256 features per core (2 heads)
HL = NH // NC       # 2 heads per core
RPC = BS // NC      # 512 output rows per core
K16 = H // 128      # 16 contraction tiles
CW = 512            # row-chunk width
QC = 512            # attention q-chunk width
SCALE = 1.0 / float(np.sqrt(HD))

_CACHED = None


def _build():
    nc = bacc.Bacc("TRN2", target_bir_lowering=False, debug=False, num_devices=NC)

    xT_d = nc.dram_tensor("xT_t", [128, K16, BS], BF16, kind="ExternalInput")
    wqT_d = nc.dram_tensor("wqT_t", [128, K16, FL], BF16, kind="ExternalInput")
    wkT_d = nc.dram_tensor("wkT_t", [128, K16, FL], BF16, kind="ExternalInput")
    wvT_d = nc.dram_tensor("wvT_t", [128, K16, FL], BF16, kind="ExternalInput")
    bq_d = nc.dram_tensor("bq", [128, HL], F32, kind="ExternalInput")
    bk_d = nc.dram_tensor("bk", [128, HL], F32, kind="ExternalInput")
    bv_d = nc.dram_tensor("bv_bc", [128, FL], F32, kind="ExternalInput")
    woT_d = nc.dram_tensor("woT_t", [128, K16, H], BF16, kind="ExternalInput")
    bo_d = nc.dram_tensor("bo_bc", [128, H], BF16, kind="ExternalInput")
    onesb_d = nc.dram_tensor("ones_bf", [128, 128], BF16, kind="ExternalInput")
    out_d = nc.dram_tensor("out", [RPC, H], F32, kind="ExternalOutput")

    with tile.TileContext(nc) as tc:
        with (
            tc.tile_pool(name="consts", bufs=1) as cstp,
            tc.tile_pool(name="dram", bufs=1, space="DRAM") as dp,
            tc.tile_pool(name="qkv", bufs=1) as qkvp,
            tc.tile_pool(name="wo01", bufs=1) as wop,
            tc.tile_pool(name="attn", bufs=1) as ap_,
            tc.tile_pool(name="psum", bufs=1, space="PSUM") as pp,
        ):
            ones_bf = cstp.tile([128, 128], BF16)
            bq_sb = cstp.tile([128, HL], F32)
            bk_sb = cstp.tile([128, HL], F32)
            bv_sb = cstp.tile([128, FL], F32)
            bo_sb = cstp.tile([128, H], BF16)

            a2a_in = [dp.tile([NC, 128, RPC], BF16, name=f"a2a_in{h}") for h in range(HL)]
            a2a_out = [dp.tile([NC, 128, RPC], BF16, name=f"a2a_out{h}") for h in range(HL)]

            qT_sb = qkvp.tile([128, HL * BS], BF16)
            kT_sb = qkvp.tile([128, HL * BS], BF16)
            v_sb = qkvp.tile([128, (BS // 128) * FL], BF16)

            won_tiles = {}

            def load_wo(pool, n):
                won = pool.tile([128, K16 * 512], BF16, tag="won", bufs=2)
                nc.sync.dma_start(won[:], woT_d.ap()[:, :, n * 512:(n + 1) * 512])
                won_tiles[n] = won

            # ---------------- projection building blocks ----------------
            def emit_proj_qk(w_sb, b_sb, dst, c, m):
                """One [128 feats x 512 rows] output block of q/k for chunk c."""
                xc = x_tiles[c]
                ps = pp.tile([128, CW], F32, tag="pa", bufs=3)
                for k in range(K16):
                    nc.tensor.matmul(
                        ps[:],
                        w_sb[:, k * FL + m * 128: k * FL + (m + 1) * 128],
                        xc[:, k * CW:(k + 1) * CW],
                        start=(k == 0),
                        stop=(k == K16 - 1),
                    )
                nc.vector.tensor_scalar_add(
                    dst[:, m * BS + c * CW: m * BS + (c + 1) * CW],
                    ps[:],
                    b_sb[:, m:m + 1],
                )

            def emit_proj_v(c, m2):
                """One [128 rows x 256 feats] block of v for chunk c."""
                xc = x_tiles[c]
                ps = pp.tile([128, CW], F32, tag="pa", bufs=3)
                for k in range(K16):
                    nc.tensor.matmul(
                        ps[:, :FL],
                        xc[:, k * CW + m2 * 128: k * CW + (m2 + 1) * 128],
                        wv_sb[:, k * FL:(k + 1) * FL],
                        start=(k == 0),
                        stop=(k == K16 - 1),
                    )
                i = c * (CW // 128) + m2
                nc.vector.tensor_add(
                    v_sb[:, i * FL:(i + 1) * FL], ps[:, :FL], bv_sb[:]
                )

            # ---------------- attention building blocks ----------------
            pend = {}

            def emit_scores_quad(key, h, b, qc, quad):
                """4 score MMs into a 4-bank PSUM tile + one wide exp."""
                base = h * BS + b * S
                if quad == 0:
                    pend[key] = ap_.tile(
                        [128, K16 * QC], BF16, tag="expT", bufs=2, name="expT"
                    )
                expT = pend[key]
                pss = pp.tile([128, 2048], F32, tag="pss", bufs=1)
                for j in range(4):
                    km = quad * 4 + j
                    nc.tensor.matmul(
                        pss[:, j * QC:(j + 1) * QC],
                        kT_sb[:, base + km * 128: base + (km + 1) * 128],
                        qT_sb[:, base + qc * QC: base + (qc + 1) * QC],
                        start=True,
                        stop=True,
                    )
                nc.scalar.activation(
                    expT[:, quad * 4 * QC:(quad + 1) * 4 * QC],
                    pss[:],
                    mybir.ActivationFunctionType.Exp,
                    scale=SCALE,
                )

            psa_pend = {}

            def emit_pv_half(key, h, b, half):
                """Half of the PV accumulation (8 of 16 k-tiles)."""
                expT = pend[key]
                if half == 0:
                    psa_pend[key] = pp.tile(
                        [128, QC], F32, tag="pa", bufs=3, name="psa"
                    )
                psa = psa_pend[key]
                for j in range(8):
                    km = half * 8 + j
                    nc.tensor.matmul(
                        psa[:],
                        v_sb[:, (16 * b + km) * FL + h * 128:
                             (16 * b + km) * FL + (h + 1) * 128],
                        expT[:, km * QC:(km + 1) * QC],
                        start=(km == 0),
                        stop=(km == K16 - 1),
                    )

            def emit_norm(key, h, b, qc):
                """Denominator tree, normalize, ship to the a2a buffer."""
                dest = b * (S // QC) + qc
                expT = pend.pop(key)
                psa = psa_pend.pop(key)
                s2 = ap_.tile([128, 4 * QC], BF16, tag="s2", bufs=1)
                nc.vector.tensor_add(s2[:], expT[:, :4 * QC], expT[:, 4 * QC:8 * QC])
                nc.vector.tensor_add(s2[:], s2[:], expT[:, 8 * QC:12 * QC])
                nc.vector.tensor_add(s2[:], s2[:], expT[:, 12 * QC:])
                s3 = ap_.tile([128, 2 * QC], BF16, tag="s3", bufs=1)
                nc.vector.tensor_add(s3[:], s2[:, :2 * QC], s2[:, 2 * QC:])
                s4 = ap_.tile([128, QC], BF16, tag="s4", bufs=2)
                nc.vector.tensor_add(s4[:], s3[:, :QC], s3[:, QC:])
                # broadcast column sums to all 128 partitions in one MM
                psum_bc = pp.tile([128, QC], F32, tag="pbc", bufs=1)
                nc.tensor.matmul(psum_bc[:], ones_bf[:], s4[:], start=True, stop=True)
                rb = ap_.tile([128, QC], F32, tag="rb", bufs=2)
                nc.vector.reciprocal_approx_fast(rb[:], psum_bc[:])
                att = ap_.tile([128, QC], BF16, tag="att", bufs=2)
                nc.vector.tensor_mul(att[:], psa[:], rb[:])
                nc.gpsimd.dma_start(a2a_in[h][dest, :, :], att[:])

            def emit_pv_norm(key, h, b, qc):
                emit_pv_half(key, h, b, 0)
                emit_pv_half(key, h, b, 1)
                emit_norm(key, h, b, qc)

            # ---------------- stages 1+2 (x/w pools open) ----------------
            with (
                tc.tile_pool(name="wgt", bufs=1) as wp,
                tc.tile_pool(name="xbf", bufs=1) as xbp,
            ):
                wq_sb = wp.tile([128, K16 * FL], BF16, tag="wq")
                wk_sb = wp.tile([128, K16 * FL], BF16, tag="wk")
                wv_sb = wp.tile([128, K16 * FL], BF16, tag="wv")

                x_tiles = {}

                def load_x(c):
                    xc = xbp.tile([128, K16 * CW], BF16, tag="x", bufs=2)
                    nc.sync.dma_start(xc[:], xT_d.ap()[:, :, c * CW:(c + 1) * CW])
                    x_tiles[c] = xc

                # Critical-path-ordered first loads: interleave wq/x0
                # quarters so the first matmuls start as early as possible.
                xc0 = xbp.tile([128, K16 * CW], BF16, tag="x", bufs=2, name="xc")
                x_tiles[0] = xc0
                for p in range(4):
                    nc.sync.dma_start(
                        wq_sb[:, p * 4 * FL:(p + 1) * 4 * FL],
                        wqT_d.ap()[:, p * 4:(p + 1) * 4, :],
                    )
                    nc.sync.dma_start(
                        xc0[:, p * 4 * CW:(p + 1) * 4 * CW],
                        xT_d.ap()[:, p * 4:(p + 1) * 4, :CW],
                    )
                nc.sync.dma_start(bq_sb[:], bq_d.ap()[:])
                nc.sync.dma_start(bk_sb[:], bk_d.ap()[:])
                nc.sync.dma_start(wk_sb[:], wkT_d.ap()[:])
                load_x(1)
                nc.sync.dma_start(wv_sb[:], wvT_d.ap()[:])
                nc.sync.dma_start(bv_sb[:], bv_d.ap()[:])
                nc.sync.dma_start(ones_bf[:], onesb_d.ap()[:])

                # Stage 1: chunks 0-3 (batch 0)
                for c in range(4):
                    if 2 <= c + 1 < 4:
                        load_x(c + 1)
                    for m in range(HL):
                        emit_proj_qk(wq_sb, bq_sb, qT_sb, c, m)
                        emit_proj_qk(wk_sb, bk_sb, kT_sb, c, m)
                    for m2 in range(CW // 128):
                        emit_proj_v(c, m2)

                # Stage 2: chunks 4-7 interleaved with attention (h0, b0, *)
                load_x(4)
                nc.sync.dma_start(bo_sb[:], bo_d.ap()[:])
                for i in range(4):
                    c = 4 + i
                    if c + 1 < 8:
                        load_x(c + 1)
                    if i < 2:
                        load_wo(wop, i)
                    key = (0, 0, i)
                    emit_scores_quad(key, 0, 0, i, 0)
                    emit_proj_qk(wq_sb, bq_sb, qT_sb, c, 0)
                    emit_scores_quad(key, 0, 0, i, 1)
                    emit_proj_qk(wq_sb, bq_sb, qT_sb, c, 1)
                    emit_scores_quad(key, 0, 0, i, 2)
                    emit_proj_qk(wk_sb, bk_sb, kT_sb, c, 0)
                    emit_scores_quad(key, 0, 0, i, 3)
                    emit_proj_qk(wk_sb, bk_sb, kT_sb, c, 1)
                    emit_pv_norm(key, 0, 0, i)
                    for m2 in range(CW // 128):
                        emit_proj_v(c, m2)

            # ---------------- stages 3+4 ----------------
            with (
                tc.tile_pool(name="wo23", bufs=1) as wop2,
                tc.tile_pool(name="aTp", bufs=1) as atp,
                tc.tile_pool(name="cpart", bufs=1) as cpp,
                tc.tile_pool(name="outC", bufs=1) as ocp,
            ):
                aT = atp.tile([128, K16 * RPC], BF16)
                partials = cpp.tile([128, 16 * 512], BF16)
                ctiles = [(n, m) for n in range(4) for m in range(4)]

                def emit_c_even(t):
                    n, m = ctiles[t]
                    won = won_tiles[n]
                    pso = pp.tile([128, 512], F32, tag="pa", bufs=3)
                    for j in range(8):
                        k = 2 * j
                        nc.tensor.matmul(
                            pso[:],
                            aT[:, k * RPC + m * 128: k * RPC + (m + 1) * 128],
                            won[:, k * 512:(k + 1) * 512],
                            start=(j == 0),
                            stop=(j == 7),
                        )
                    # stash evens + bias as a bf16 partial
                    nc.vector.tensor_add(
                        partials[:, t * 512:(t + 1) * 512],
                        pso[:],
                        bo_sb[:, n * 512:(n + 1) * 512],
                    )

                def emit_c_odd(t):
                    n, m = ctiles[t]
                    won = won_tiles[n]
                    pso = pp.tile([128, 512], F32, tag="pa", bufs=3)
                    for j in range(8):
                        k = 2 * j + 1
                        nc.tensor.matmul(
                            pso[:],
                            aT[:, k * RPC + m * 128: k * RPC + (m + 1) * 128],
                            won[:, k * 512:(k + 1) * 512],
                            start=(j == 0),
                            stop=(j == 7),
                        )
                    ot = ocp.tile([128, 512], F32, tag="ot", bufs=3)
                    nc.vector.tensor_add(
                        ot[:], pso[:], partials[:, t * 512:(t + 1) * 512]
                    )
                    nc.sync.dma_start(
                        out_d.ap()[m * 128:(m + 1) * 128, n * 512:(n + 1) * 512],
                        ot[:],
                    )

                # Stage 3: 1-deep software pipeline — chunk i+1's score
                # quads are woven between chunk i's PV halves so the PE
                # never waits on ACT.  Order: (h0,b1,*) -> A2A(h0);
                # (h1,b0,*), (h1,b1,*) -> A2A(h1).  A few C-even pre-runs
                # interleave late; the bulk fills the A2A(h1) window.
                s3_chunks = [(0, 1, qc) for qc in range(4)] + [
                    (1, b, qc) for b in range(B) for qc in range(4)
                ]

                def fire_a2a(h):
                    nc.gpsimd.collective_compute(
                        "AllToAll",
                        mybir.AluOpType.bypass,
                        ins=[a2a_in[h].opt()],
                        outs=[a2a_out[h].opt()],
                        replica_groups=[list(range(NC))],
                    )
                    for g in range(h, K16, 2):
                        nc.sync.dma_start(
                            aT[:, g * RPC:(g + 1) * RPC], a2a_out[h][g // 2, :, :]
                        )

                prev = None
                for idx, (h, b, qc) in enumerate(s3_chunks):
                    if idx < 2:
                        load_wo(wop2, 2 + idx)
                    key = (h, b, qc)
                    emit_scores_quad(key, h, b, qc, 0)
                    if prev is not None:
                        emit_pv_half(prev, prev[0], prev[1], 0)
                    emit_scores_quad(key, h, b, qc, 1)
                    if prev is not None:
                        emit_pv_half(prev, prev[0], prev[1], 1)
                    emit_scores_quad(key, h, b, qc, 2)
                    if prev is not None:
                        emit_norm(prev, prev[0], prev[1], prev[2])
                        if prev == (0, 1, 3):
                            fire_a2a(0)
                    emit_scores_quad(key, h, b, qc, 3)
                    if idx >= 10:
                        emit_c_even(2 * (idx - 10))
                        emit_c_even(2 * (idx - 10) + 1)
                    prev = key
                emit_pv_norm(prev, prev[0], prev[1], prev[2])
                fire_a2a(1)
                for t in range(4, 16):
                    emit_c_even(t)

                # Stage 4: odd halves + combine
                for t in range(16):
                    emit_c_odd(t)

    nc.compile()
    return nc


def _get_nc():
    global _CACHED
    if _CACHED is None:
        _CACHED = _build()
    return _CACHED


def _prep_in_maps(x, Wq, bq, Wk, bk, Wv, bv, Wo, bo):
    import ml_dtypes

    bf = ml_dtypes.bfloat16

    def tile_kmaj(a2d):
        # [H, N] -> [128, K16, N] with row r = k*128 + p
        h, n = a2d.shape
        return np.ascontiguousarray(
            a2d.reshape(K16, 128, n).transpose(1, 0, 2).astype(bf)
        )

    xT_t = tile_kmaj(x.reshape(BS, H).T)
    woT_t = tile_kmaj(Wo.T)
    bo_bc = np.ascontiguousarray(np.broadcast_to(bo, (128, H)).astype(bf))
    ones_bf = np.ones((128, 128), bf)
    in_maps = []
    for c in range(NC):
        sl = slice(FL * c, FL * (c + 1))
        in_maps.append(
            {
                "xT_t": xT_t,
                "wqT_t": tile_kmaj(np.ascontiguousarray(Wq[sl, :].T)),
                "wkT_t": tile_kmaj(np.ascontiguousarray(Wk[sl, :].T)),
                "wvT_t": tile_kmaj(np.ascontiguousarray(Wv[sl, :].T)),
                "bq": np.ascontiguousarray(bq[sl].reshape(HL, 128).T),
                "bk": np.ascontiguousarray(bk[sl].reshape(HL, 128).T),
                "bv_bc": np.ascontiguousarray(np.broadcast_to(bv[sl], (128, FL))),
                "woT_t": woT_t,
                "bo_bc": bo_bc,
                "ones_bf": ones_bf,
            }
        )
    return in_maps


def run(in_maps, trace=False):
    nc = _get_nc()
    return run_bass_kernel_spmd(nc, in_maps, core_ids=list(range(NC)), trace=trace)


def kernel(x, Wq, bq, Wk, bk, Wv, bv, Wo, bo):
    args = [np.asarray(a, dtype=np.float32) for a in (x, Wq, bq, Wk, bk, Wv, bv, Wo, bo)]
    in_maps = _prep_in_maps(*args)
    res = run(in_maps)
    out = np.concatenate([res.results[c]["out"] for c in range(NC)], axis=0)
    return out.reshape(B, S, H)


# revision 14
# speedup vs baseline: 1.1544x; 1.0319x over previous
"""Multi-head self-attention (no mask) on 8 TRN2 NeuronCores.

Sharding: tensor-parallel over heads (2 heads/core) for QKV + attention,
then an AllToAll re-shards to row-parallel for the output projection.

v2 structure (fused schedule, all inputs pre-cast to bf16 on host):
  Stage 1: QKV projections for row-chunks 0-3 (batch 0).
  Stage 2: projections for chunks 4-7 (batch 1) interleaved with
     attention chunks (h0, b0, *) so the PE covers ACT's exp latency.
  Stage 3: remaining 12 attention chunks; AllToAll(h0) fires 1/3 in;
     out-projection even-k-slab pre-runs (stashed to SBUF bf16 partials)
     fill the AllToAll(h1) window.
  Stage 4: odd-k-slab accumulation + partial add + bias, stream out.

Attention chunk: scores into a [128,2048] 4-bank PSUM tile (4 MMs), one
wide exp ACTIVATE per quad; PV accumulates v^T expT; softmax denominators
via DVE pairwise tree + a ones[128x128] matmul that broadcasts the sums
to all partitions in one shot; reciprocal_approx_fast + one DVE mul
normalizes. No max-subtraction (scores are O(5)).
"""

import numpy as np

import concourse.bass as bass
import concourse.tile as tile
from concourse import bacc, mybir
from concourse.bass_utils import run_bass_kernel_spmd

F32 = mybir.dt.float32
BF16 = mybir.dt.bfloat16

B, S, H = 2, 2048, 2048
NH, HD = 16, 128
NC = 8
BS = B * S          # 4096 rows total
FL = H // NC        # 256 features per core (2 heads)
HL = NH // NC       # 2 heads per core
RPC = BS // NC      # 512 output rows per core
K16 = H // 128      # 16 contraction tiles
CW = 512            # row-chunk width
QC = 512            # attention q-chunk width
SCALE = 1.0 / float(np.sqrt(HD))

_CACHED = None


def _build():
    nc = bacc.Bacc("TRN2", target_bir_lowering=False, debug=False, num_devices=NC)

    xT_d = nc.dram_tensor("xT_t", [128, K16, BS], BF16, kind="ExternalInput")
    wqT_d = nc.dram_tensor("wqT_t", [128, K16, FL], BF16, kind="ExternalInput")
    wkT_d = nc.dram_tensor("wkT_t", [128, K16, FL], BF16, kind="ExternalInput")
    wvT_d = nc.dram_tensor("wvT_t", [128, K16, FL], BF16, kind="ExternalInput")
    bq_d = nc.dram_tensor("bq", [128, HL], F32, kind="ExternalInput")
    bk_d = nc.dram_tensor("bk", [128, HL], F32, kind="ExternalInput")
    bv_d = nc.dram_tensor("bv_bc", [128, FL], F32, kind="ExternalInput")
    woT_d = nc.dram_tensor("woT_t", [128, K16, H], BF16, kind="ExternalInput")
    bo_d = nc.dram_tensor("bo_bc", [128, H], BF16, kind="ExternalInput")
    onesb_d = nc.dram_tensor("ones_bf", [128, 128], BF16, kind="ExternalInput")
    out_d = nc.dram_tensor("out", [RPC, H], F32, kind="ExternalOutput")

    with tile.TileContext(nc) as tc:
        with (
            tc.tile_pool(name="consts", bufs=1) as cstp,
            tc.tile_pool(name="dram", bufs=1, space="DRAM") as dp,
            tc.tile_pool(name="qkv", bufs=1) as qkvp,
            tc.tile_pool(name="wo01", bufs=1) as wop,
            tc.tile_pool(name="attn", bufs=1) as ap_,
            tc.tile_pool(name="psum", bufs=1, space="PSUM") as pp,
        ):
            ones_bf = cstp.tile([128, 128], BF16)
            bq_sb = cstp.tile([128, HL], F32)
            bk_sb = cstp.tile([128, HL], F32)
            bv_sb = cstp.tile([128, FL], F32)
            bo_sb = cstp.tile([128, H], BF16)

            a2a_in = [dp.tile([NC, 128, RPC], BF16, name=f"a2a_in{h}") for h in range(HL)]
            a2a_out = [dp.tile([NC, 128, RPC], BF16, name=f"a2a_out{h}") for h in range(HL)]

            qT_sb = qkvp.tile([128, HL * BS], BF16)
            kT_sb = qkvp.tile([128, HL * BS], BF16)
            v_sb = qkvp.tile([128, (BS // 128) * FL], BF16)

            won_tiles = {}

            def load_wo(pool, n):
                won = pool.tile([128, K16 * 512], BF16, tag="won", bufs=2)
                nc.sync.dma_start(won[:], woT_d.ap()[:, :, n * 512:(n + 1) * 512])
                won_tiles[n] = won

            # ---------------- projection building blocks ----------------
            def emit_proj_qk(w_sb, b_sb, dst, c, m):
                """One [128 feats x 512 rows] output block of q/k for chunk c."""
                xc = x_tiles[c]
                ps = pp.tile([128, CW], F32, tag="pa", bufs=3)
                for k in range(K16):
                    nc.tensor.matmul(
                        ps[:],
                        w_sb[:, k * FL + m * 128: k * FL + (m + 1) * 128],
                        xc[:, k * CW:(k + 1) * CW],
                        start=(k == 0),
                        stop=(k == K16 - 1),
                    )
                nc.vector.tensor_scalar_add(
                    dst[:, m * BS + c * CW: m * BS + (c + 1) * CW],
                    ps[:],
                    b_sb[:, m:m + 1],
                )

            def emit_proj_v(c, m2):
                """One [128 rows x 256 feats] block of v for chunk c."""
                xc = x_tiles[c]
                ps = pp.tile([128, CW], F32, tag="pa", bufs=3)
                for k in range(K16):
                    nc.tensor.matmul(
                        ps[:, :FL],
                        xc[:, k * CW + m2 * 128: k * CW + (m2 + 1) * 128],
                        wv_sb[:, k * FL:(k + 1) * FL],
                        start=(k == 0),
                        stop=(k == K16 - 1),
                    )
                i = c * (CW // 128) + m2
                nc.vector.tensor_add(
                    v_sb[:, i * FL:(i + 1) * FL], ps[:, :FL], bv_sb[:]
                )

            # ---------------- attention building blocks ----------------
            pend = {}

            def emit_scores_quad(key, h, b, qc, quad):
                """4 score MMs into a 4-bank PSUM tile + one wide exp."""
                base = h * BS + b * S
                if quad == 0:
                    pend[key] = ap_.tile(
                        [128, K16 * QC], BF16, tag="expT", bufs=2, name="expT"
                    )
                expT = pend[key]
                pss = pp.tile([128, 2048], F32, tag="pss", bufs=1)
                for j in range(4):
                    km = quad * 4 + j
                    nc.tensor.matmul(
                        pss[:, j * QC:(j + 1) * QC],
                        kT_sb[:, base + km * 128: base + (km + 1) * 128],
                        qT_sb[:, base + qc * QC: base + (qc + 1) * QC],
                        start=True,
                        stop=True,
                    )
                nc.scalar.activation(
                    expT[:, quad * 4 * QC:(quad + 1) * 4 * QC],
                    pss[:],
                    mybir.ActivationFunctionType.Exp,
                    scale=SCALE,
                )

            psa_pend = {}

            def emit_pv_half(key, h, b, half):
                """Half of the PV accumulation (8 of 16 k-tiles)."""
                expT = pend[key]
                if half == 0:
                    psa_pend[key] = pp.tile(
                        [128, QC], F32, tag="pa", bufs=3, name="psa"
                    )
                psa = psa_pend[key]
                for j in range(8):
                    km = half * 8 + j
                    nc.tensor.matmul(
                        psa[:],
                        v_sb[:, (16 * b + km) * FL + h * 128:
                             (16 * b + km) * FL + (h + 1) * 128],
                        expT[:, km * QC:(km + 1) * QC],
                        start=(km == 0),
                        stop=(km == K16 - 1),
                    )

            def emit_norm(key, h, b, qc):
                """Denominator tree, normalize, ship to the a2a buffer."""
                dest = b * (S // QC) + qc
                expT = pend.pop(key)
                psa = psa_pend.pop(key)
                s2 = ap_.tile([128, 4 * QC], BF16, tag="s2", bufs=1)
                nc.vector.tensor_add(s2[:], expT[:, :4 * QC], expT[:, 4 * QC:8 * QC])
                nc.vector.tensor_add(s2[:], s2[:], expT[:, 8 * QC:12 * QC])
                nc.vector.tensor_add(s2[:], s2[:], expT[:, 12 * QC:])
                s3 = ap_.tile([128, 2 * QC], BF16, tag="s3", bufs=1)
                nc.vector.tensor_add(s3[:], s2[:, :2 * QC], s2[:, 2 * QC:])
                s4 = ap_.tile([128, QC], BF16, tag="s4", bufs=2)
                nc.vector.tensor_add(s4[:], s3[:, :QC], s3[:, QC:])
                # broadcast column sums to all 128 partitions in one MM
                psum_bc = pp.tile([128, QC], F32, tag="pbc", bufs=1)
                nc.tensor.matmul(psum_bc[:], ones_bf[:], s4[:], start=True, stop=True)
                rb = ap_.tile([128, QC], F32, tag="rb", bufs=2)
                nc.vector.reciprocal_approx_fast(rb[:], psum_bc[:])
                att = ap_.tile([128, QC], BF16, tag="att", bufs=2)
                nc.vector.tensor_mul(att[:], psa[:], rb[:])
                nc.gpsimd.dma_start(a2a_in[h][dest, :, :], att[:])

            def emit_pv_norm(key, h, b, qc):
                emit_pv_half(key, h, b, 0)
                emit_pv_half(key, h, b, 1)
                emit_norm(key, h, b, qc)

            # ---------------- stages 1+2 (x/w pools open) ----------------
            with (
                tc.tile_pool(name="wgt", bufs=1) as wp,
                tc.tile_pool(name="xbf", bufs=1) as xbp,
            ):
                wq_sb = wp.tile([128, K16 * FL], BF16, tag="wq")
                wk_sb = wp.tile([128, K16 * FL], BF16, tag="wk")
                wv_sb = wp.tile([128, K16 * FL], BF16, tag="wv")

                x_tiles = {}

                def load_x(c):
                    xc = xbp.tile([128, K16 * CW], BF16, tag="x", bufs=2)
                    nc.sync.dma_start(xc[:], xT_d.ap()[:, :, c * CW:(c + 1) * CW])
                    x_tiles[c] = xc

                # Critical-path-ordered first loads: interleave wq/x0
                # quarters so the first matmuls start as early as possible.
                xc0 = xbp.tile([128, K16 * CW], BF16, tag="x", bufs=2, name="xc")
                x_tiles[0] = xc0
                for p in range(4):
                    nc.sync.dma_start(
                        wq_sb[:, p * 4 * FL:(p + 1) * 4 * FL],
                        wqT_d.ap()[:, p * 4:(p + 1) * 4, :],
                    )
                    nc.sync.dma_start(
                        xc0[:, p * 4 * CW:(p + 1) * 4 * CW],
                        xT_d.ap()[:, p * 4:(p + 1) * 4, :CW],
                    )
                nc.sync.dma_start(bq_sb[:], bq_d.ap()[:])
                nc.sync.dma_start(bk_sb[:], bk_d.ap()[:])
                nc.sync.dma_start(wk_sb[:], wkT_d.ap()[:])
                load_x(1)
                nc.sync.dma_start(wv_sb[:], wvT_d.ap()[:])
                nc.sync.dma_start(bv_sb[:], bv_d.ap()[:])
                nc.sync.dma_start(ones_bf[:], onesb_d.ap()[:])

                # Stage 1: chunks 0-3 (batch 0)
                for c in range(4):
                    if 2 <= c + 1 < 4:
                        load_x(c + 1)
                    for m in range(HL):
                        emit_proj_qk(wq_sb, bq_sb, qT_sb, c, m)
                        emit_proj_qk(wk_sb, bk_sb, kT_sb, c, m)
                    for m2 in range(CW // 128):
                        emit_proj_v(c, m2)

                # Stage 2: chunks 4-7 interleaved with attention (h0, b0, *)
                load_x(4)
                nc.sync.dma_start(bo_sb[:], bo_d.ap()[:])
                for i in range(4):
                    c = 4 + i
                    if c + 1 < 8:
                        load_x(c + 1)
                    if i < 2:
                        load_wo(wop, i)
                    key = (0, 0, i)
                    emit_scores_quad(key, 0, 0, i, 0)
                    emit_proj_qk(wq_sb, bq_sb, qT_sb, c, 0)
                    emit_scores_quad(key, 0, 0, i, 1)
                    emit_proj_qk(wq_sb, bq_sb, qT_sb, c, 1)
                    emit_scores_quad(key, 0, 0, i, 2)
                    emit_proj_qk(wk_sb, bk_sb, kT_sb, c, 0)
                    emit_scores_quad(key, 0, 0, i, 3)
                    emit_proj_qk(wk_sb, bk_sb, kT_sb, c, 1)
                    if i < 3:
                        emit_pv_norm(key, 0, 0, i)
                    for m2 in range(CW // 128):
                        emit_proj_v(c, m2)

            # ---------------- stages 3+4 ----------------
            with (
                tc.tile_pool(name="wo23", bufs=1) as wop2,
                tc.tile_pool(name="aTp", bufs=1) as atp,
                tc.tile_pool(name="cpart", bufs=1) as cpp,
                tc.tile_pool(name="outC", bufs=1) as ocp,
            ):
                aT = atp.tile([128, K16 * RPC], BF16)
                partials = cpp.tile([128, 16 * 512], BF16)
                ctiles = [(n, m) for n in range(4) for m in range(4)]

                def emit_c_even(t):
                    n, m = ctiles[t]
                    won = won_tiles[n]
                    pso = pp.tile([128, 512], F32, tag="pa", bufs=3)
                    for j in range(8):
                        k = 2 * j
                        nc.tensor.matmul(
                            pso[:],
                            aT[:, k * RPC + m * 128: k * RPC + (m + 1) * 128],
                            won[:, k * 512:(k + 1) * 512],
                            start=(j == 0),
                            stop=(j == 7),
                        )
                    # stash evens + bias as a bf16 partial
                    nc.vector.tensor_add(
                        partials[:, t * 512:(t + 1) * 512],
                        pso[:],
                        bo_sb[:, n * 512:(n + 1) * 512],
                    )

                def emit_c_odd(t):
                    n, m = ctiles[t]
                    won = won_tiles[n]
                    pso = pp.tile([128, 512], F32, tag="pa", bufs=3)
                    for j in range(8):
                        k = 2 * j + 1
                        nc.tensor.matmul(
                            pso[:],
                            aT[:, k * RPC + m * 128: k * RPC + (m + 1) * 128],
                            won[:, k * 512:(k + 1) * 512],
                            start=(j == 0),
                            stop=(j == 7),
                        )
                    ot = ocp.tile([128, 512], F32, tag="ot", bufs=3)
                    nc.vector.tensor_add(
                        ot[:], pso[:], partials[:, t * 512:(t + 1) * 512]
                    )
                    nc.sync.dma_start(
                        out_d.ap()[m * 128:(m + 1) * 128, n * 512:(n + 1) * 512],
                        ot[:],
                    )

                # Stage 3: 1-deep software pipeline — chunk i+1's score
                # quads are woven between chunk i's PV halves so the PE
                # never waits on ACT.  Order: (h0,b1,*) -> A2A(h0);
                # (h1,b0,*), (h1,b1,*) -> A2A(h1).  A few C-even pre-runs
                # interleave late; the bulk fills the A2A(h1) window.
                s3_chunks = [(0, 1, qc) for qc in range(4)] + [
                    (1, b, qc) for b in range(B) for qc in range(4)
                ]

                def fire_a2a(h):
                    nc.gpsimd.collective_compute(
                        "AllToAll",
                        mybir.AluOpType.bypass,
                        ins=[a2a_in[h].opt()],
                        outs=[a2a_out[h].opt()],
                        replica_groups=[list(range(NC))],
                    )
                    for g in range(h, K16, 2):
                        nc.sync.dma_start(
                            aT[:, g * RPC:(g + 1) * RPC], a2a_out[h][g // 2, :, :]
                        )

                prev = (0, 0, 3)
                for idx, (h, b, qc) in enumerate(s3_chunks):
                    if idx < 2:
                        load_wo(wop2, 2 + idx)
                    key = (h, b, qc)
                    emit_scores_quad(key, h, b, qc, 0)
                    emit_pv_half(prev, prev[0], prev[1], 0)
                    emit_scores_quad(key, h, b, qc, 1)
                    emit_pv_half(prev, prev[0], prev[1], 1)
                    emit_scores_quad(key, h, b, qc, 2)
                    emit_norm(prev, prev[0], prev[1], prev[2])
                    if prev == (0, 1, 3):
                        fire_a2a(0)
                    emit_scores_quad(key, h, b, qc, 3)
                    prev = key
                emit_pv_norm(prev, prev[0], prev[1], prev[2])
                fire_a2a(1)
                for t in range(16):
                    emit_c_even(t)

                # Stage 4: odd halves + combine
                for t in range(16):
                    emit_c_odd(t)

    nc.compile()
    return nc


def _get_nc():
    global _CACHED
    if _CACHED is None:
        _CACHED = _build()
    return _CACHED


def _prep_in_maps(x, Wq, bq, Wk, bk, Wv, bv, Wo, bo):
    import ml_dtypes

    bf = ml_dtypes.bfloat16

    def tile_kmaj(a2d):
        # [H, N] -> [128, K16, N] with row r = k*128 + p
        h, n = a2d.shape
        return np.ascontiguousarray(
            a2d.reshape(K16, 128, n).transpose(1, 0, 2).astype(bf)
        )

    xT_t = tile_kmaj(x.reshape(BS, H).T)
    woT_t = tile_kmaj(Wo.T)
    bo_bc = np.ascontiguousarray(np.broadcast_to(bo, (128, H)).astype(bf))
    ones_bf = np.ones((128, 128), bf)
    in_maps = []
    for c in range(NC):
        sl = slice(FL * c, FL * (c + 1))
        in_maps.append(
            {
                "xT_t": xT_t,
                "wqT_t": tile_kmaj(np.ascontiguousarray(Wq[sl, :].T)),
                "wkT_t": tile_kmaj(np.ascontiguousarray(Wk[sl, :].T)),
                "wvT_t": tile_kmaj(np.ascontiguousarray(Wv[sl, :].T)),
                "bq": np.ascontiguousarray(bq[sl].reshape(HL, 128).T),
                "bk": np.ascontiguousarray(bk[sl].reshape(HL, 128).T),
                "bv_bc": np.ascontiguousarray(np.broadcast_to(bv[sl], (128, FL))),
                "woT_t": woT_t,
                "bo_bc": bo_bc,
                "ones_bf": ones_bf,
            }
        )
    return in_maps


def run(in_maps, trace=False):
    nc = _get_nc()
    return run_bass_kernel_spmd(nc, in_maps, core_ids=list(range(NC)), trace=trace)


def kernel(x, Wq, bq, Wk, bk, Wv, bv, Wo, bo):
    args = [np.asarray(a, dtype=np.float32) for a in (x, Wq, bq, Wk, bk, Wv, bv, Wo, bo)]
    in_maps = _prep_in_maps(*args)
    res = run(in_maps)
    out = np.concatenate([res.results[c]["out"] for c in range(NC)], axis=0)
    return out.reshape(B, S, H)
